# revision 13
# baseline (speedup 1.0000x reference)
"""Classwise-ECE (segmentation) kernel for 8 Trainium2 NeuronCores.

Math: with conf = softmax(logits, axis=C) laid out [C, N] and y = 15*conf,
the reference ECE reduces to
    sce = mean_c sum_b |D[c,b]| / N,
    D[c,b] = conf_sum[c,b] - acc_sum[c,b]
where conf_sum[c,b] = sum of conf over pixels with y in bin (b, b+1], and
acc_sum[c,b] = count of pixels with label==c whose y_c falls in that bin.

Device scheme (per core, pixels sharded 8 ways):
  layout [114 = 6 slots x 19 classes, NF] pixels on the free axis.
  et = bf16(exp(bf16 logits))                       [ACT]
  S  = per-(slot,pixel) sums of et over 19 classes  [PE block-ones matmul]
  rpk = 1/S                                         [DVE reciprocal]
  rb  = 15/S broadcast to 114 rows (w2 holds 15.0)  [PE matmul -> PSUM]
  rs  = bf16(rb)                                    [ACT copy]
  y   = bf16(et * rs)                               [DVE tensor_tensor @2x]
  z   = bf16(labeq * y)                             [Pool tensor_tensor]
  then per-row accumulators via tensor_scalar+accum_out (bf16 SBUF = 4x
  DVE mode; scalar_tensor_tensor has no perf modes, which is why the old
  14x stt kernel was 2x slower at the same pass count):
    T       = sum y                      1 pass  (DVE @4x)
    clamp_b = sum clamp(y,b,b+1)        14 passes (DVE @4x)
    ylo_t   = #{y <= t}                 14 passes (DVE @4x, minus ACT set)
    zlo_t   = #{z <= t}                 14 passes (DVE @4x, minus Pool set)
  ACT_T thresholds run on ACT as sum sign(y - t - 1/512) with accum
  (exact: bf16 y never equals t + 1/512); POOL_ZT run on GPSIMD.
Host algebra (f64): SY_b = clamp_b - b*ylo_b - (b+1)*(NF - ylo_{b+1});
SY_0 = T - sum SY_b; conf_sum = SY/15; acc from zlo differences with
#{z==0} known from labels; pad pixels subtracted exactly.
"""

import numpy as np

C = 19
NB = 15
SLOTS = 6
P = SLOTS * C            # 114 partitions
FD = 512                 # pixels per PE chunk
B, H, W = 4, 512, 1024
N = B * H * W
N_CORES = 8
NPC = N // N_CORES       # 262144 pixels per core
CHUNKS = -(-NPC // (SLOTS * FD))   # 86
NF = CHUNKS * FD         # 44032 pixels per slot row
NPIX = SLOTS * NF
NPAD = NPIX - NPC        # 2048 zero-logit pad pixels per core
GROUP = 3                # chunks per S-pack PSUM tile (32-row spacing)
SROWS = 32 * (GROUP - 1) + SLOTS   # 70 packed S partitions
GFD = 15 * FD            # 7680 pixels per outer group
NGROUPS = -(-NF // GFD)  # 6 (last group 5632)

# threshold engine assignment (t values 1..14)
ACT_T = (11, 12, 13, 14)     # ylo thresholds computed on ACT via Sign
POOL_ZT = ()                 # TensorScalarPtr+reduce is illegal on Pool
NQ = 1 + (NB - 1) * 3        # 43 accumulator quantities per group

_CACHE = {}


def _group_cols():
    out = []
    c0 = 0
    while c0 < NF:
        out.append((c0, min(GFD, NF - c0)))
        c0 += out[-1][1]
    return out


def _build_program():
    from contextlib import ExitStack
    import concourse.bass as bass
    import concourse.tile as tile
    from concourse import bacc, mybir

    f32 = mybir.dt.float32
    bf16 = mybir.dt.bfloat16
    ALU = mybir.AluOpType
    ACTF = mybir.ActivationFunctionType

    nc = bacc.Bacc("TRN2", target_bir_lowering=False, debug=False,
                   num_devices=N_CORES)

    lg = nc.dram_tensor("lg", [P, NF], bf16, kind="ExternalInput").ap()
    le = nc.dram_tensor("le", [P, NF], bf16, kind="ExternalInput").ap()
    w1 = nc.dram_tensor("w1", [P, GROUP * SROWS], bf16,
                        kind="ExternalInput").ap()
    w2 = nc.dram_tensor("w2", [SROWS, P], f32, kind="ExternalInput").ap()
    accd = nc.dram_tensor("acc", [P, NQ * NGROUPS], f32,
                          kind="ExternalOutput").ap()

    groups = _group_cols()

    with tile.TileContext(nc) as tc, ExitStack() as ctx:
        const_pool = ctx.enter_context(tc.tile_pool(name="const", bufs=1))
        lg_pool = ctx.enter_context(tc.tile_pool(name="lgp", bufs=2))
        le_pool = ctx.enter_context(tc.tile_pool(name="lep", bufs=2))
        et_pool = ctx.enter_context(tc.tile_pool(name="etp", bufs=1))
        y_pool = ctx.enter_context(tc.tile_pool(name="yp", bufs=2))
        z_pool = ctx.enter_context(tc.tile_pool(name="zp", bufs=1))
        rs_pool = ctx.enter_context(tc.tile_pool(name="rsp", bufs=1))
        rpk_pool = ctx.enter_context(tc.tile_pool(name="rpk", bufs=6))
        ps_s = ctx.enter_context(
            tc.tile_pool(name="ps_s", bufs=2, space=bass.MemorySpace.PSUM))
        ps_rb = ctx.enter_context(
            tc.tile_pool(name="ps_rb", bufs=2, space=bass.MemorySpace.PSUM))

        w1_sb = const_pool.tile([P, GROUP * SROWS], bf16)
        nc.sync.dma_start(w1_sb[:], w1)
        w2_sb = const_pool.tile([SROWS, P], f32)
        nc.sync.dma_start(w2_sb[:], w2)
        acc = const_pool.tile([P, NQ * NGROUPS], f32)
        tr_d = const_pool.tile([P, GFD], bf16)   # DVE trash out
        tr_a = const_pool.tile([P, GFD], bf16)   # ACT trash out
        tr_p = const_pool.tile([P, GFD], bf16)   # Pool trash out
        abias = const_pool.tile([P, len(ACT_T)], f32)
        for k, t in enumerate(ACT_T):
            nc.gpsimd.memset(abias[:, k:k + 1], -(t + 1.0 / 512.0))

        for g, (c0, gw) in enumerate(groups):
            nch = gw // FD
            lgt = lg_pool.tile([P, gw], bf16, tag="lg")
            nc.sync.dma_start(lgt[:], lg[:, c0:c0 + gw])
            let = le_pool.tile([P, gw], bf16, tag="le")
            nc.sync.dma_start(let[:], le[:, c0:c0 + gw])

            et = et_pool.tile([P, gw], bf16, tag="et")
            nc.scalar.activation(et[:], lgt[:], ACTF.Exp)

            yt = y_pool.tile([P, gw], bf16, tag="y")
            rst = rs_pool.tile([P, gw], bf16, tag="rs")

            for j0 in range(0, nch, GROUP):
                js = list(range(j0, min(j0 + GROUP, nch)))
                spack = ps_s.tile([SROWS, FD], f32, tag="spack")
                for q, j in enumerate(js):
                    nc.tensor.matmul(
                        spack[:],
                        w1_sb[:, q * SROWS:(q + 1) * SROWS],
                        et[:, j * FD:(j + 1) * FD],
                        start=(q == 0), stop=(q == len(js) - 1))
                rpk = rpk_pool.tile([SROWS, FD], f32, tag="rpk")
                nc.vector.reciprocal(rpk[:], spack[:])
                rbw = len(js) * FD
                rb = ps_rb.tile([P, rbw], f32, tag="rb")
                for q, j in enumerate(js):
                    nc.tensor.matmul(
                        rb[:, q * FD:(q + 1) * FD],
                        w2_sb[32 * q:32 * q + SLOTS, :],
                        rpk[32 * q:32 * q + SLOTS, :],
                        start=True, stop=True)
                sl = slice(j0 * FD, j0 * FD + rbw)
                nc.scalar.activation(rst[:, sl], rb[:], ACTF.Copy)
                nc.vector.tensor_mul(yt[:, sl], et[:, sl], rst[:, sl])

            zt = z_pool.tile([P, gw], bf16, tag="z")
            nc.gpsimd.tensor_mul(zt[:], let[:], yt[:])

            col = g * NQ
            # T = sum y   (accum_out: op1 is the REDUCE op; op0 elementwise)
            nc.vector.tensor_scalar(tr_d[:, :gw], yt[:], 1.0, 0.0,
                                    op0=ALU.mult, op1=ALU.add,
                                    accum_out=acc[:, col:col + 1])
            # M_t = sum max(y, t): G_t = M_t - t*ylo_t, SY_b = G_b - G_{b+1}
            for b in range(1, NB):
                cc = col + b
                nc.vector.tensor_scalar(tr_d[:, :gw], yt[:],
                                        float(b), 0.0,
                                        op0=ALU.max, op1=ALU.add,
                                        accum_out=acc[:, cc:cc + 1])
            # ylo counts
            for t in range(1, NB):
                cc = col + 14 + t
                if t in ACT_T:
                    k = ACT_T.index(t)
                    nc.scalar.activation(tr_a[:, :gw], yt[:], ACTF.Sign,
                                         bias=abias[:, k:k + 1], scale=1.0,
                                         accum_out=acc[:, cc:cc + 1])
                else:
                    nc.vector.tensor_scalar(tr_d[:, :gw], yt[:],
                                            float(t), 0.0, op0=ALU.is_le,
                                            op1=ALU.add,
                                            accum_out=acc[:, cc:cc + 1])
            # zlo counts
            for t in range(1, NB):
                cc = col + 28 + t
                eng = nc.gpsimd if t in POOL_ZT else nc.vector
                tr = tr_p if t in POOL_ZT else tr_d
                eng.tensor_scalar(tr[:, :gw], zt[:], float(t), 0.0,
                                  op0=ALU.is_le, op1=ALU.add,
                                  accum_out=acc[:, cc:cc + 1])

        nc.sync.dma_start(accd, acc[:])

    nc.compile()
    return nc


def _get_program():
    if "nc" not in _CACHE:
        _CACHE["nc"] = _build_program()
    return _CACHE["nc"]


def _host_constants():
    import ml_dtypes
    w1 = np.zeros((P, GROUP * SROWS), ml_dtypes.bfloat16)
    w2 = np.zeros((SROWS, P), np.float32)
    for s in range(SLOTS):
        for c in range(C):
            p = s * C + c
            for j in range(GROUP):
                w1[p, j * SROWS + 32 * j + s] = 1.0
                w2[32 * j + s, p] = np.float32(15.0)
    return w1, w2


def kernel(logits, labels, _trace=False):
    import ml_dtypes
    from concourse.bass_utils import run_bass_kernel_spmd

    bf16 = ml_dtypes.bfloat16
    logits = np.asarray(logits, dtype=np.float32)
    labels = np.asarray(labels)
    lt = np.moveaxis(logits, 1, 0).reshape(C, N)
    lf = labels.reshape(N).astype(np.int32)

    w1, w2 = _host_constants()
    cids = np.arange(C, dtype=np.int32)
    in_maps = []
    nlab_rows = []
    for i in range(N_CORES):
        sl = slice(i * NPC, (i + 1) * NPC)
        lgc = np.zeros((C, NPIX), np.float32)
        lgc[:, :NPC] = lt[:, sl]
        lgc = np.ascontiguousarray(
            lgc.reshape(C, SLOTS, NF).transpose(1, 0, 2).reshape(P, NF)
        ).astype(bf16)
        lbc = np.zeros((NPIX,), np.int32)
        lbc[:NPC] = lf[sl]
        lec = (lbc.reshape(SLOTS, 1, NF) == cids[None, :, None])
        nlab_rows.append(lec.reshape(P, NF).sum(axis=1))
        lec = np.ascontiguousarray(lec.reshape(P, NF).astype(bf16))
        in_maps.append({"lg": lgc, "le": lec, "w1": w1, "w2": w2})

    nc = _get_program()
    res = run_bass_kernel_spmd(nc, in_maps, list(range(N_CORES)),
                               trace=_trace)
    _CACHE["last_exec_ns"] = res.exec_time_ns
    _CACHE["last_trace"] = res.instructions_and_trace
    _CACHE["last_profile_json"] = res.profile_json

    groups = _group_cols()
    conf_sum = np.zeros((C, NB))
    acc_sum = np.zeros((C, NB))
    for i, r in enumerate(res.results):
        a = r["acc"].astype(np.float64).reshape(P, NGROUPS, NQ)
        T = a[:, :, 0].sum(axis=1)
        M = np.zeros((P, NB))
        ylo = np.zeros((P, NB))
        zlo = np.zeros((P, NB))
        for b in range(1, NB):
            M[:, b] = a[:, :, b].sum(axis=1)
        for t in range(1, NB):
            v = a[:, :, 14 + t]
            if t in ACT_T:
                # v holds per-group sum sign(y - t') = Nck - 2*ylo
                sizes = np.array([gw for (_, gw) in groups], np.float64)
                ylo[:, t] = (sizes[None, :] - v).sum(axis=1) / 2.0
            else:
                ylo[:, t] = v.sum(axis=1)
            zlo[:, t] = a[:, :, 28 + t].sum(axis=1)

        # G_t = sum_{y>t} y = M_t - t*ylo_t; SY_b = G_b - G_{b+1}
        G = np.zeros((P, NB + 1))
        for t in range(1, NB):
            G[:, t] = M[:, t] - t * ylo[:, t]
        SY = np.zeros((P, NB))
        for b in range(1, NB):
            SY[:, b] = G[:, b] - G[:, b + 1]
        SY[:, 0] = T - G[:, 1]
        AC = np.zeros((P, NB))
        nz = NF - nlab_rows[i]            # #{z==0} per row
        AC[:, 0] = zlo[:, 1] - nz
        for b in range(1, NB - 1):
            AC[:, b] = zlo[:, b + 1] - zlo[:, b]
        AC[:, NB - 1] = NF - zlo[:, NB - 1]

        conf_sum += SY.reshape(SLOTS, C, NB).sum(axis=0) / NB
        acc_sum += AC.reshape(SLOTS, C, NB).sum(axis=0)

    # pad pixels: logits 0 -> et 1, S = 19, y_pad = bf16(bf16(15/19)*1)
    y_pad = float(bf16(np.float32(15.0) * (np.float32(1.0) /
                                           np.float32(19.0))))
    pad_total = NPAD * N_CORES
    conf_sum[:, 0] -= pad_total * y_pad / NB
    acc_sum[0, 0] -= pad_total

    D = conf_sum - acc_sum
    sce = np.abs(D).sum(axis=1).mean() / N
    return np.float32(sce)


# revision 17
# speedup vs baseline: 2.9202x; 2.9202x over previous
"""Classwise-ECE (segmentation) kernel for 8 Trainium2 NeuronCores.

Math: with conf = softmax(logits, axis=C) laid out [C, N] and y = 15*conf,
the reference ECE reduces to
    sce = mean_c sum_b |D[c,b]| / N,
    D[c,b] = conf_sum[c,b] - acc_sum[c,b]
where conf_sum[c,b] = sum of conf over pixels with y in bin (b, b+1], and
acc_sum[c,b] = count of pixels with label==c whose y_c falls in that bin.

Device scheme (per core, pixels sharded 8 ways):
  layout [114 = 6 slots x 19 classes, NF] pixels on the free axis.
  et = bf16(exp(bf16 logits))                       [ACT]
  S  = per-(slot,pixel) sums of et over 19 classes  [PE block-ones matmul]
  rpk = 1/S                                         [DVE reciprocal]
  rb  = 15/S broadcast to 114 rows (w2 holds 15.0)  [PE matmul -> PSUM]
  rs  = bf16(rb)                                    [ACT copy]
  y   = bf16(et * rs)                               [DVE tensor_tensor @2x]
  z   = bf16(labeq * y)                             [Pool tensor_tensor]
  then per-row accumulators via tensor_scalar+accum_out (bf16 SBUF = 4x
  DVE mode; scalar_tensor_tensor has no perf modes, which is why the old
  14x stt kernel was 2x slower at the same pass count):
    T       = sum y                      1 pass  (DVE @4x)
    clamp_b = sum clamp(y,b,b+1)        14 passes (DVE @4x)
    ylo_t   = #{y <= t}                 14 passes (DVE @4x, minus ACT set)
    zlo_t   = #{z <= t}                 14 passes (DVE @4x, minus Pool set)
  ACT_T thresholds run on ACT as sum sign(y - t - 1/512) with accum
  (exact: bf16 y never equals t + 1/512); POOL_ZT run on GPSIMD.
Host algebra (f64): SY_b = clamp_b - b*ylo_b - (b+1)*(NF - ylo_{b+1});
SY_0 = T - sum SY_b; conf_sum = SY/15; acc from zlo differences with
#{z==0} known from labels; pad pixels subtracted exactly.
"""

import numpy as np

C = 19
NB = 15
SLOTS = 6
P = SLOTS * C            # 114 partitions
FD = 512                 # pixels per PE chunk
B, H, W = 4, 512, 1024
N = B * H * W
N_CORES = 8
NPC = N // N_CORES       # 262144 pixels per core
CHUNKS = -(-NPC // (SLOTS * FD))   # 86
NF = CHUNKS * FD         # 44032 pixels per slot row
NPIX = SLOTS * NF
NPAD = NPIX - NPC        # 2048 zero-logit pad pixels per core
GROUP = 3                # chunks per S-pack PSUM tile (32-row spacing)
SROWS = 32 * (GROUP - 1) + SLOTS   # 70 packed S partitions
GFD = 15 * FD            # 7680 pixels per outer group
NGROUPS = -(-NF // GFD)  # 6 (last group 5632)

# Engine split: DVE TensorScalarPtrReduce has only 1x uops (8.06us per
# 7680-elem pass at 0.96GHz); ACT's activation accumulator does the same
# reduction in (224+FD)/1.2GHz = 6.6us. Balance ~half/half.
# G-family: DVE computes M_t = sum max(y,t); ACT computes R_t = sum Relu(y-t).
# counts: DVE is_le; ACT Sign(y - t - 1/512) (exact for bf16 y).
ACT_G = (1, 2, 3, 4, 5, 6, 7, 8, 9)     # G-source via ACT Relu
ACT_YLO = (10, 11, 12, 13, 14)          # ylo via ACT Sign
ACT_ZLO = (9, 10, 11, 12, 13, 14)       # zlo via ACT Sign
T_ON_ACT = True                          # T = sum y via ACT Copy accum
NQ = 1 + (NB - 1) * 3        # 43 accumulator quantities per group

_CACHE = {}


def _group_cols():
    out = []
    c0 = 0
    while c0 < NF:
        out.append((c0, min(GFD, NF - c0)))
        c0 += out[-1][1]
    return out


def _build_program():
    from contextlib import ExitStack
    import concourse.bass as bass
    import concourse.tile as tile
    from concourse import bacc, mybir

    f32 = mybir.dt.float32
    bf16 = mybir.dt.bfloat16
    ALU = mybir.AluOpType
    ACTF = mybir.ActivationFunctionType

    nc = bacc.Bacc("TRN2", target_bir_lowering=False, debug=False,
                   num_devices=N_CORES)

    lg = nc.dram_tensor("lg", [P, NF], bf16, kind="ExternalInput").ap()
    le = nc.dram_tensor("le", [P, NF], bf16, kind="ExternalInput").ap()
    w1 = nc.dram_tensor("w1", [P, GROUP * SROWS], bf16,
                        kind="ExternalInput").ap()
    w2 = nc.dram_tensor("w2", [SROWS, P], f32, kind="ExternalInput").ap()
    accd = nc.dram_tensor("acc", [P, NQ * NGROUPS], f32,
                          kind="ExternalOutput").ap()

    groups = _group_cols()

    with tile.TileContext(nc) as tc, ExitStack() as ctx:
        const_pool = ctx.enter_context(tc.tile_pool(name="const", bufs=1))
        lg_pool = ctx.enter_context(tc.tile_pool(name="lgp", bufs=2))
        le_pool = ctx.enter_context(tc.tile_pool(name="lep", bufs=2))
        et_pool = ctx.enter_context(tc.tile_pool(name="etp", bufs=1))
        y_pool = ctx.enter_context(tc.tile_pool(name="yp", bufs=2))
        z_pool = ctx.enter_context(tc.tile_pool(name="zp", bufs=1))
        rs_pool = ctx.enter_context(tc.tile_pool(name="rsp", bufs=1))
        rpk_pool = ctx.enter_context(tc.tile_pool(name="rpk", bufs=6))
        ps_s = ctx.enter_context(
            tc.tile_pool(name="ps_s", bufs=2, space=bass.MemorySpace.PSUM))
        ps_rb = ctx.enter_context(
            tc.tile_pool(name="ps_rb", bufs=2, space=bass.MemorySpace.PSUM))

        w1_sb = const_pool.tile([P, GROUP * SROWS], bf16)
        nc.sync.dma_start(w1_sb[:], w1)
        w2_sb = const_pool.tile([SROWS, P], f32)
        nc.sync.dma_start(w2_sb[:], w2)
        acc = const_pool.tile([P, NQ * NGROUPS], f32)
        tr_d = const_pool.tile([P, GFD], bf16)   # DVE trash out
        tr_a = const_pool.tile([P, GFD], bf16)   # ACT trash out
        # bias columns: Relu biases -t for ACT_G, Sign biases -(t+1/512)
        bias_vals = ([float(-t) for t in ACT_G]
                     + [-(t + 1.0 / 512.0)
                        for t in sorted(set(ACT_YLO) | set(ACT_ZLO))])
        bcol = {v: k for k, v in enumerate(bias_vals)}
        abias = const_pool.tile([P, len(bias_vals)], f32)
        for v, k in bcol.items():
            nc.gpsimd.memset(abias[:, k:k + 1], v)

        for g, (c0, gw) in enumerate(groups):
            nch = gw // FD
            lgt = lg_pool.tile([P, gw], bf16, tag="lg")
            nc.sync.dma_start(lgt[:], lg[:, c0:c0 + gw])
            let = le_pool.tile([P, gw], bf16, tag="le")
            nc.sync.dma_start(let[:], le[:, c0:c0 + gw])

            et = et_pool.tile([P, gw], bf16, tag="et")
            nc.scalar.activation(et[:], lgt[:], ACTF.Exp)

            yt = y_pool.tile([P, gw], bf16, tag="y")
            rst = rs_pool.tile([P, gw], bf16, tag="rs")

            for j0 in range(0, nch, GROUP):
                js = list(range(j0, min(j0 + GROUP, nch)))
                spack = ps_s.tile([SROWS, FD], f32, tag="spack")
                for q, j in enumerate(js):
                    nc.tensor.matmul(
                        spack[:],
                        w1_sb[:, q * SROWS:(q + 1) * SROWS],
                        et[:, j * FD:(j + 1) * FD],
                        start=(q == 0), stop=(q == len(js) - 1))
                rpk = rpk_pool.tile([SROWS, FD], f32, tag="rpk")
                nc.vector.reciprocal(rpk[:], spack[:])
                rbw = len(js) * FD
                rb = ps_rb.tile([P, rbw], f32, tag="rb")
                for q, j in enumerate(js):
                    nc.tensor.matmul(
                        rb[:, q * FD:(q + 1) * FD],
                        w2_sb[32 * q:32 * q + SLOTS, :],
                        rpk[32 * q:32 * q + SLOTS, :],
                        start=True, stop=True)
                sl = slice(j0 * FD, j0 * FD + rbw)
                nc.scalar.activation(rst[:, sl], rb[:], ACTF.Copy)
                nc.vector.tensor_mul(yt[:, sl], et[:, sl], rst[:, sl])

            zt = z_pool.tile([P, gw], bf16, tag="z")
            nc.gpsimd.tensor_mul(zt[:], let[:], yt[:])

            col = g * NQ
            # T = sum y   (accum_out: op1 is the REDUCE op; op0 elementwise)
            if T_ON_ACT:
                nc.scalar.activation(tr_a[:, :gw], yt[:], ACTF.Copy,
                                     bias=0.0, scale=1.0,
                                     accum_out=acc[:, col:col + 1])
            else:
                nc.vector.tensor_scalar(tr_d[:, :gw], yt[:], 1.0, 0.0,
                                        op0=ALU.mult, op1=ALU.add,
                                        accum_out=acc[:, col:col + 1])
            # G-family: DVE M_t = sum max(y,t); ACT R_t = sum Relu(y-t)
            for b in range(1, NB):
                cc = col + b
                if b in ACT_G:
                    k = bcol[float(-b)]
                    nc.scalar.activation(tr_a[:, :gw], yt[:], ACTF.Relu,
                                         bias=abias[:, k:k + 1], scale=1.0,
                                         accum_out=acc[:, cc:cc + 1])
                else:
                    nc.vector.tensor_scalar(tr_d[:, :gw], yt[:],
                                            float(b), 0.0,
                                            op0=ALU.max, op1=ALU.add,
                                            accum_out=acc[:, cc:cc + 1])
            # ylo counts
            for t in range(1, NB):
                cc = col + 14 + t
                if t in ACT_YLO:
                    k = bcol[-(t + 1.0 / 512.0)]
                    nc.scalar.activation(tr_a[:, :gw], yt[:], ACTF.Sign,
                                         bias=abias[:, k:k + 1], scale=1.0,
                                         accum_out=acc[:, cc:cc + 1])
                else:
                    nc.vector.tensor_scalar(tr_d[:, :gw], yt[:],
                                            float(t), 0.0, op0=ALU.is_le,
                                            op1=ALU.add,
                                            accum_out=acc[:, cc:cc + 1])
            # zlo counts
            for t in range(1, NB):
                cc = col + 28 + t
                if t in ACT_ZLO:
                    k = bcol[-(t + 1.0 / 512.0)]
                    nc.scalar.activation(tr_a[:, :gw], zt[:], ACTF.Sign,
                                         bias=abias[:, k:k + 1], scale=1.0,
                                         accum_out=acc[:, cc:cc + 1])
                else:
                    nc.vector.tensor_scalar(tr_d[:, :gw], zt[:],
                                            float(t), 0.0,
                                            op0=ALU.is_le, op1=ALU.add,
                                            accum_out=acc[:, cc:cc + 1])

        nc.sync.dma_start(accd, acc[:])

    nc.compile()
    return nc


def _get_program():
    if "nc" not in _CACHE:
        _CACHE["nc"] = _build_program()
    return _CACHE["nc"]


def _host_constants():
    import ml_dtypes
    w1 = np.zeros((P, GROUP * SROWS), ml_dtypes.bfloat16)
    w2 = np.zeros((SROWS, P), np.float32)
    for s in range(SLOTS):
        for c in range(C):
            p = s * C + c
            for j in range(GROUP):
                w1[p, j * SROWS + 32 * j + s] = 1.0
                w2[32 * j + s, p] = np.float32(15.0)
    return w1, w2


def kernel(logits, labels, _trace=False):
    import ml_dtypes
    from concourse.bass_utils import run_bass_kernel_spmd

    bf16 = ml_dtypes.bfloat16
    logits = np.asarray(logits, dtype=np.float32)
    labels = np.asarray(labels)
    lt = np.moveaxis(logits, 1, 0).reshape(C, N)
    lf = labels.reshape(N).astype(np.int32)

    w1, w2 = _host_constants()
    cids = np.arange(C, dtype=np.int32)
    in_maps = []
    nlab_rows = []
    for i in range(N_CORES):
        sl = slice(i * NPC, (i + 1) * NPC)
        lgc = np.zeros((C, NPIX), np.float32)
        lgc[:, :NPC] = lt[:, sl]
        lgc = np.ascontiguousarray(
            lgc.reshape(C, SLOTS, NF).transpose(1, 0, 2).reshape(P, NF)
        ).astype(bf16)
        lbc = np.zeros((NPIX,), np.int32)
        lbc[:NPC] = lf[sl]
        lec = (lbc.reshape(SLOTS, 1, NF) == cids[None, :, None])
        nlab_rows.append(lec.reshape(P, NF).sum(axis=1))
        lec = np.ascontiguousarray(lec.reshape(P, NF).astype(bf16))
        in_maps.append({"lg": lgc, "le": lec, "w1": w1, "w2": w2})

    nc = _get_program()
    res = run_bass_kernel_spmd(nc, in_maps, list(range(N_CORES)),
                               trace=_trace)
    _CACHE["last_exec_ns"] = res.exec_time_ns
    _CACHE["last_trace"] = res.instructions_and_trace
    _CACHE["last_profile_json"] = res.profile_json

    groups = _group_cols()
    conf_sum = np.zeros((C, NB))
    acc_sum = np.zeros((C, NB))
    for i, r in enumerate(res.results):
        a = r["acc"].astype(np.float64).reshape(P, NGROUPS, NQ)
        sizes = np.array([gw for (_, gw) in groups], np.float64)
        T = a[:, :, 0].sum(axis=1)
        M = np.zeros((P, NB))
        ylo = np.zeros((P, NB))
        zlo = np.zeros((P, NB))
        for b in range(1, NB):
            M[:, b] = a[:, :, b].sum(axis=1)
        for t in range(1, NB):
            v = a[:, :, 14 + t]
            if t in ACT_YLO:
                # v holds per-group sum sign(y - t') = Nck - 2*ylo
                ylo[:, t] = (sizes[None, :] - v).sum(axis=1) / 2.0
            else:
                ylo[:, t] = v.sum(axis=1)
            vz = a[:, :, 28 + t]
            if t in ACT_ZLO:
                zlo[:, t] = (sizes[None, :] - vz).sum(axis=1) / 2.0
            else:
                zlo[:, t] = vz.sum(axis=1)

        # G_t = sum_{y>t} y: DVE M_t = sum max(y,t) -> G = M - t*ylo
        #                    ACT R_t = sum relu(y-t) -> G = R + t*(NF - ylo)
        G = np.zeros((P, NB + 1))
        for t in range(1, NB):
            if t in ACT_G:
                G[:, t] = M[:, t] + t * (NF - ylo[:, t])
            else:
                G[:, t] = M[:, t] - t * ylo[:, t]
        SY = np.zeros((P, NB))
        for b in range(1, NB):
            SY[:, b] = G[:, b] - G[:, b + 1]
        SY[:, 0] = T - G[:, 1]
        AC = np.zeros((P, NB))
        nz = NF - nlab_rows[i]            # #{z==0} per row
        AC[:, 0] = zlo[:, 1] - nz
        for b in range(1, NB - 1):
            AC[:, b] = zlo[:, b + 1] - zlo[:, b]
        AC[:, NB - 1] = NF - zlo[:, NB - 1]

        conf_sum += SY.reshape(SLOTS, C, NB).sum(axis=0) / NB
        acc_sum += AC.reshape(SLOTS, C, NB).sum(axis=0)

    # pad pixels: logits 0 -> et 1, S = 19, y_pad = bf16(bf16(15/19)*1)
    y_pad = float(bf16(np.float32(15.0) * (np.float32(1.0) /
                                           np.float32(19.0))))
    pad_total = NPAD * N_CORES
    conf_sum[:, 0] -= pad_total * y_pad / NB
    acc_sum[0, 0] -= pad_total

    D = conf_sum - acc_sum
    sce = np.abs(D).sum(axis=1).mean() / N
    return np.float32(sce)
